# revision 15
# baseline (speedup 1.0000x reference)
"""Trainium2 Bass kernel for nn_KnowledgeIntegrationLoss.

Reference semantics:
    x = [new_knowledge; existing_knowledge]            # [11, 8192]
    E = MLP_encoder(x)                                 # [11, 128] (3 Linear, ReLU x2)
    geo_j = ||E_0 - E_j||, cos_j = <E_0,E_j>/(max(|E_0|,eps)*max(|E_j|,eps))
    avg = mean_{j=1..10}(geo_j - cos_j)
    q = MLP_integrator([E_0; math_metrics])            # [1]
    out = avg + exp(-q)                                # scalar

Distribution (8 NeuronCores, no device collectives — launch skew across the
axon-tunneled cores makes any cross-core dependency cost 20-250us):
  Phase 1 (SPMD x8): column-shard W1 (64 cols/core) in bf16:
      h1_j = ReLU(x @ W1[:, 64j:64j+64] + b1_j)          (col form [64, 11])
      z_j^T = W2[64j:64j+64, :].T @ h1_j                  -> [256 -> [128, 22], 11]
  Host: pure concatenation of the 8 z_j^T blocks (zero FLOPs).
  Phase 2 (1 core): sum z_j (elementwise), +b2, ReLU -> h2^T; layer 3 + the
  loss tail. sqrt/rsqrt are done on DVE via the bit-trick + Newton (avoids
  ACT table swaps: no act table holds both Exp and Sqrt); the only ACT op is
  the final Exp whose table load overlaps the input DMA.
"""

import numpy as np
import ml_dtypes

import concourse.bass as bass
import concourse.mybir as mybir
import concourse.tile as tile
from concourse import bacc
from concourse import bass_utils

F32 = mybir.dt.float32
BF16 = mybir.dt.bfloat16
I32 = mybir.dt.int32
NP_BF16 = ml_dtypes.bfloat16
N_CORES = 8
KDIM = 8192
EPS2 = 1e-16  # eps^2 so max(n2, eps^2) == max(norm, eps)^2
ALU = mybir.AluOpType
AF = mybir.ActivationFunctionType

RSQRT_MAGIC_P1 = 0x5F3759E0  # 0x5F3759DF + 1 (for the ~(x>>1) + C form)

# ---------------------------------------------------------------------------
# host-side layout helpers (pure reshapes/transposes/dtype casts, no FLOPs)
# ---------------------------------------------------------------------------


def _kmajor_image(a, p=128):
    """[K, M] (K = c*p + part) -> image [p, (K//p)*M]: img[part, c*M+m] = a[c*p+part, m]."""
    K, M = a.shape
    n = K // p
    return np.ascontiguousarray(a.reshape(n, p, M).transpose(1, 0, 2).reshape(p, n * M))


# ---------------------------------------------------------------------------
# phase 1: per-core first-layer column shard + z = h1 @ W2-shard precompute
# ---------------------------------------------------------------------------

XW_COLS = 704 + 24 + 256 + 3072  # x | fp32(id,b1) as bf16 | W2 shard | w1[0:48 tiles]
N_JUNK = 180             # HAM warm-up matmuls (PE-only, no cross-engine deps)


def build_phase1():
    nc = bacc.Bacc("TRN2", target_bir_lowering=False, debug=False,
                   num_devices=N_CORES)

    imgA = nc.dram_tensor("imgA", [128, XW_COLS], BF16, kind="ExternalInput")
    imgB = nc.dram_tensor("imgB", [128, 1024], BF16, kind="ExternalInput")
    z_out = nc.dram_tensor("z_out", [128, 22], BF16, kind="ExternalOutput")

    ost = nc.ctx.enter_context(nc.sbuf_tensor("ost", [128, 22], BF16))
    osem = nc.alloc_semaphore("osem")
    with tile.TileContext(nc) as tc:
        with (
            tc.tile_pool(name="sbuf", bufs=1) as sb,
            tc.tile_pool(name="psum", bufs=1, space="PSUM") as ps,
        ):
            # Two input DMAs on the Sync HWDGE ring (FIFO): the big image
            # (x + consts + first half of W1) first, then the W1 tail.
            xw = sb.tile([128, XW_COLS], BF16, tag="xw")
            nc.sync.dma_start(xw[:], imgA[:])
            w1b = sb.tile([128, 1024], BF16, tag="w1b")
            nc.sync.dma_start(w1b[:], imgB[:])

            idsb = xw[0:11, 704:726].bitcast(F32)    # [11, 11] identity
            b1col = xw[0:64, 726:728].bitcast(F32)   # [64, 1]
            w2sA = xw[0:64, 728:856]
            w2sB = xw[0:64, 856:984]

            # PE warm-up: cheap bf16 junk matmuls keep the HAM activity window
            # busy during the input DMA wait so the real stream runs at 2.4GHz.
            junk = sb.tile([128, 16], BF16, tag="junk")
            nc.vector.memset(junk[:], 0.0)
            jps = ps.tile([16, 16], F32, tag="jps")
            for _ in range(N_JUNK):
                nc.tensor.matmul(jps[:, :], junk[:, 0:16], junk[:, 0:16],
                                 start=True, stop=True)

            # layer-1 contraction, 2 col-group-packed accumulation chains:
            # even k-tiles -> psum1[0:11], odd k-tiles -> psum1[32:43]; the
            # packed LDWEIGHTS/MATMULs overlap across col groups.
            psum1 = ps.tile([64, 64], F32, tag="psum1")
            for n in range(64):
                w1t = xw[:, 984 + 64 * n:984 + 64 * (n + 1)] if n < 48 else \
                    w1b[:, 64 * (n - 48):64 * (n - 47)]
                c = n & 1
                nc.tensor.matmul(
                    psum1[32 * c:32 * c + 11, :],
                    xw[:, 11 * n:11 * (n + 1)],
                    w1t,
                    start=(n < 2),
                    stop=(n >= 62),
                    tile_position=(0, 32 * c),
                )

            chB = sb.tile([11, 64], F32, tag="chB")
            nc.vector.tensor_copy(chB[:], psum1[32:43, :])
            pre = sb.tile([11, 64], F32, tag="pre")
            nc.vector.tensor_add(pre[:], psum1[0:11, :], chB[:])
            psT = ps.tile([64, 11], F32, tag="psT")
            nc.tensor.transpose(psT[:], pre[:], idsb)
            # h1 = relu(pre.T + b1) in bf16 col form [64, 11]
            h1t = sb.tile([64, 11], BF16, tag="h1t")
            nc.vector.tensor_scalar(h1t[:], psT[:], b1col, 0.0, ALU.add, ALU.max)

            # z^T halves: [128, 11] = W2s_half.T @ h1  (K = 64)
            zps = ps.tile([128, 22], F32, tag="zps")
            nc.tensor.matmul(zps[:, 0:11], w2sA, h1t[0:64, :], start=True, stop=True)
            nc.tensor.matmul(zps[:, 11:22], w2sB, h1t[0:64, :], start=True, stop=True)
            # Untracked output path: copy to a raw SBUF staging buffer and
            # DMA out without a completion wait — the end-of-kernel barrier
            # then overlaps the HBM write receipt with the sem-wipe epilogue.
            nc.vector.tensor_copy(ost.ap(), zps[:])
            nc.vector.sem_inc(osem, 1)
            nc.scalar.wait_ge(osem, 1)
            nc.scalar.dma_start(z_out[:], ost.ap())
    nc.compile()
    return nc


def phase1_inputs(x, W1, b1, W2):
    """Per-core input maps. x: [11, 8192] fp32."""
    xk = _kmajor_image(np.ascontiguousarray(x.T)).astype(NP_BF16)  # [128, 704]
    f32sec = np.zeros((128, 12), np.float32)
    f32sec[0:11, 0:11] = np.eye(11, dtype=np.float32)
    maps = []
    for j in range(N_CORES):
        xw = np.zeros((128, XW_COLS), NP_BF16)
        xw[:, 0:704] = xk
        fs = f32sec.copy()
        fs[0:64, 11] = b1[64 * j:64 * (j + 1)]
        # round to bf16 precision so the low halves are zero (no spurious
        # bf16-NaN bit patterns in the packed image)
        fs = fs.astype(NP_BF16).astype(np.float32)
        xw[:, 704:728] = fs.view(np.uint16).view(NP_BF16)
        xw[0:64, 728:984] = W2[64 * j:64 * (j + 1), :].astype(NP_BF16)
        w1j = _kmajor_image(
            np.ascontiguousarray(W1[:, 64 * j:64 * (j + 1)])).astype(NP_BF16)
        xw[:, 984:4056] = w1j[:, 0:3072]
        maps.append({"imgA": xw, "imgB": np.ascontiguousarray(w1j[:, 3072:4096])})
    return maps


# ---------------------------------------------------------------------------
# phase 2: z-sum, layers 2..3 epilogue + loss tail, single core
# ---------------------------------------------------------------------------

# single bf16 image; cols 176:204 hold fp32 data (bitcast pairs):
#   z 0:176 | b2 176:180 | ones1f 180:202 | b3row 202:330 | b3c 330:331 |
#   w3 331:587 | onesbf 587:598 | wmean 598:609 | wi1a 609:673 |
#   wi1b7 673:737 | mm6e 737:738 | wi2e 738:770 | wi3e 770:771
PIMG_COLS = 772  # even so fp32 bitcast views stride cleanly


def build_phase2():
    nc = bacc.Bacc("TRN2", target_bir_lowering=False, debug=False, num_devices=1)

    pimg = nc.dram_tensor("pimg", [128, PIMG_COLS], BF16, kind="ExternalInput")
    out = nc.dram_tensor("out", [1, 1], F32, kind="ExternalOutput")

    ost = nc.ctx.enter_context(nc.sbuf_tensor("ost", [1, 1], F32))
    osem = nc.alloc_semaphore("osem")
    with tile.TileContext(nc) as tc:
        with (
            tc.tile_pool(name="sbuf", bufs=1) as sb,
            tc.tile_pool(name="psum", bufs=1, space="PSUM") as ps,
        ):
            psb = sb.tile([128, PIMG_COLS], BF16, tag="psb")
            nc.sync.dma_start(psb[:], pimg[:])

            b2c = [psb[:, 176 + 2 * h:178 + 2 * h].bitcast(F32) for h in range(2)]
            ones1f = psb[0:1, 180:202].bitcast(F32)
            b3row = psb[0:11, 202:330]
            b3c = psb[:, 330:331]
            w3sb = psb[:, 331:587]
            onesbf = psb[0:1, 587:598]
            wmean = psb[0:11, 598:599]
            wi1a = psb[:, 609:673]
            wi1b7 = psb[0:7, 673:737]
            mm6e = psb[0:7, 737:738]
            wi2e = psb[0:65, 738:770]
            wi3e = psb[0:33, 770:771]

            # hidden vectors with a trailing 1.0 partition (bias via K-extension)
            i1r = sb.tile([65, 1], BF16, tag="i1r")
            nc.vector.memset(i1r[64:65, :], 1.0)
            i2r = sb.tile([33, 1], BF16, tag="i2r")
            nc.vector.memset(i2r[32:33, :], 1.0)

            # ---- z sum (8 blocks of [128, 22]) via 3 rounds of wide adds
            s1t = sb.tile([128, 88], F32, tag="s1t")
            nc.vector.tensor_add(s1t[:], psb[:, 0:88], psb[:, 88:176])
            s2t = sb.tile([128, 44], F32, tag="s2t")
            nc.vector.tensor_add(s2t[:], s1t[:, 0:44], s1t[:, 44:88])
            s3t = sb.tile([128, 22], F32, tag="s3t")
            nc.vector.tensor_add(s3t[:], s2t[:, 0:22], s2t[:, 22:44])

            # h2^T = relu(z + b2) in bf16 [128, 22]
            h2t = sb.tile([128, 22], BF16, tag="h2t")
            for h in range(2):
                nc.vector.tensor_scalar(h2t[:, 11 * h:11 * (h + 1)],
                                        s3t[:, 11 * h:11 * (h + 1)],
                                        b2c[h], 0.0, ALU.add, ALU.max)

            # ---- layer 3 row form: E = h2 @ W3 (no bias yet) -> [11, 128]
            psum3 = ps.tile([11, 128], F32, tag="p3")
            for h in range(2):
                nc.tensor.matmul(
                    psum3[:, :], h2t[:, 11 * h:11 * (h + 1)],
                    w3sb[:, 128 * h:128 * (h + 1)],
                    start=(h == 0), stop=(h == 1),
                )
            Esb = sb.tile([11, 128], BF16, tag="Esb")
            nc.vector.tensor_add(Esb[:], psum3[:], b3row)

            # ---- layer 3 col form: E^T [128, 11]; col 0 feeds the integrator
            psET = ps.tile([128, 11], F32, tag="pET")
            for h in range(2):
                nc.tensor.matmul(
                    psET[:, :], w3sb[:, 128 * h:128 * (h + 1)],
                    h2t[:, 11 * h:11 * (h + 1)],
                    start=(h == 0), stop=(h == 1),
                )
            newT = sb.tile([128, 1], BF16, tag="newT")
            nc.vector.tensor_add(newT[:], psET[:, 0:1], b3c)

            # ---- integrator MLP on [E_0; math_metrics], column form
            i1c = ps.tile([64, 1], F32, tag="pi1")
            nc.tensor.matmul(i1c[:, :], wi1a, newT[:, 0:1], start=True, stop=False)
            nc.tensor.matmul(i1c[:, :], wi1b7, mm6e, start=False, stop=True)
            nc.vector.tensor_scalar_max(i1r[0:64, :], i1c[:, :], 0.0)
            i2c = ps.tile([32, 1], F32, tag="pi2")
            nc.tensor.matmul(i2c[:, :], wi2e, i1r[:, 0:1], start=True, stop=True)
            nc.vector.tensor_scalar_max(i2r[0:32, :], i2c[:, :], 0.0)
            qp = ps.tile([1, 1], F32, tag="pq")
            nc.tensor.matmul(qp[:, :], wi3e, i2r[:, 0:1], start=True, stop=True)
            il = sb.tile([1, 1], F32, tag="il")
            nc.scalar.activation(il[:], qp[:], AF.Exp, scale=-1.0)

            # ---- broadcast row 0 of E to all 11 partitions (PE ones trick)
            bcN = ps.tile([11, 128], F32, tag="pbc")
            nc.tensor.matmul(bcN[:, :], onesbf, Esb[0:1, :], start=True, stop=True)

            # ---- row reductions: n2_j = |E_j|^2, dv_j = <E_j, E_0>
            pk = sb.tile([11, 2], F32, tag="pk")   # [g2 | n2]
            scr = sb.tile([11, 128], BF16, tag="scr")
            nc.vector.scalar_tensor_tensor(
                out=scr[:], in0=Esb[:], scalar=1.0, in1=Esb[:],
                op0=ALU.bypass, op1=ALU.mult, accum_out=pk[:, 1:2])
            dv = sb.tile([11, 1], F32, tag="dv")
            scr2 = sb.tile([11, 128], BF16, tag="scr2")
            nc.vector.scalar_tensor_tensor(
                out=scr2[:], in0=Esb[:], scalar=1.0, in1=bcN[:],
                op0=ALU.bypass, op1=ALU.mult, accum_out=dv[:])

            # bc0_j = n2_0 for all j (PE broadcast of pk[0, 1])
            bc0 = ps.tile([11, 1], F32, tag="pb0")
            nc.tensor.matmul(bc0[:, :], ones1f, pk[0:1, 1:2], start=True, stop=True)
            # g2 = n2_0 + n2 - 2*dv
            t4 = sb.tile([11, 1], F32, tag="t4")
            nc.vector.tensor_add(t4[:], bc0[:], pk[:, 1:2])
            nc.vector.scalar_tensor_tensor(
                out=pk[:, 0:1], in0=dv[:], scalar=-2.0, in1=t4[:],
                op0=ALU.mult, op1=ALU.add)

            # ---- rsqrt on both columns (bit-trick seed + 1 fused Newton
            # step); x = 0 (the j=0 geo slot) stays finite and lands on 0.
            y = sb.tile([11, 2], F32, tag="y")
            si = sb.tile([11, 2], I32, tag="si")
            nc.vector.tensor_scalar(si[:], pk[:].bitcast(I32), 1, -1,
                                    ALU.arith_shift_right, ALU.bitwise_xor)
            nc.vector.tensor_scalar_add(y[:].bitcast(I32), si[:], RSQRT_MAGIC_P1)
            u = sb.tile([11, 2], F32, tag="u")
            w = sb.tile([11, 2], F32, tag="w")
            nc.vector.tensor_mul(u[:], y[:], y[:])
            nc.vector.scalar_tensor_tensor(
                out=w[:], in0=u[:], scalar=-0.5, in1=pk[:],
                op0=ALU.mult, op1=ALU.mult)
            nc.vector.scalar_tensor_tensor(
                out=y[:], in0=w[:], scalar=1.5, in1=y[:],
                op0=ALU.add, op1=ALU.mult)

            # gc = [geo | cosw]: geo = g2 * rsqrt(g2); cosw = dv * rsqrt(n2)
            gc = sb.tile([11, 2], BF16, tag="gc")
            nc.vector.tensor_mul(gc[:, 0:1], pk[:, 0:1], y[:, 0:1])
            nc.vector.tensor_mul(gc[:, 1:2], dv[:], y[:, 1:2])

            # s12 = wmean^T @ [geo | cosw]  (wmean = 0, -0.1 x10)
            s12 = ps.tile([1, 2], F32, tag="ps12")
            nc.tensor.matmul(s12[:, :], wmean, gc[:, :], start=True, stop=True)

            # mean(geo - cos) = rn0 * s12[1] - s12[0]   (rn0 = rsqrt(n2_0))
            s12sb = sb.tile([1, 2], F32, tag="s12sb")
            nc.vector.tensor_copy(s12sb[:], s12[:])
            m1 = sb.tile([1, 1], F32, tag="m1")
            nc.vector.scalar_tensor_tensor(
                out=m1[:], in0=s12sb[0:1, 1:2], scalar=y[0:1, 1:2],
                in1=s12sb[0:1, 0:1], op0=ALU.mult, op1=ALU.subtract)
            # Untracked output path (see phase 1).
            nc.vector.tensor_add(ost.ap(), m1[:], il[:])
            nc.vector.sem_inc(osem, 1)
            nc.sync.wait_ge(osem, 1)
            nc.sync.dma_start(out[:], ost.ap())
    nc.compile()
    return nc


def phase2_input_maps(z_full, b2, W3, b3, Wi1, bi1, Wi2, bi2, Wi3, bi3,
                      math_metrics):
    """z_full: [128, 176] bf16 = hstack of the 8 per-core phase-1 outputs."""
    img = np.zeros((128, PIMG_COLS), NP_BF16)
    img[:, 0:176] = z_full
    f32 = np.zeros((128, 13), np.float32)
    f32[:, 0:2] = b2.reshape(2, 128).T
    f32[0, 2:13] = 1.0
    f32 = f32.astype(NP_BF16).astype(np.float32)
    img[:, 176:202] = f32.view(np.uint16).view(NP_BF16)
    img[0:11, 202:330] = np.tile(b3, (11, 1)).astype(NP_BF16)
    img[:, 330] = b3.astype(NP_BF16)
    img[:, 331:587] = _kmajor_image(W3).astype(NP_BF16)
    img[0, 587:598] = np.ones(11, NP_BF16)
    img[1:11, 598] = NP_BF16(-0.1)
    img[:, 609:673] = Wi1[:128].astype(NP_BF16)
    img[0:7, 673:737] = np.concatenate([Wi1[128:], bi1.reshape(1, 64)],
                                       axis=0).astype(NP_BF16)
    img[0:7, 737] = np.concatenate([math_metrics,
                                    np.ones(1, np.float32)]).astype(NP_BF16)
    img[0:65, 738:770] = np.concatenate([Wi2, bi2.reshape(1, 32)],
                                        axis=0).astype(NP_BF16)
    img[0:33, 770] = np.concatenate([Wi3.reshape(-1), bi3]).astype(NP_BF16)
    return {"pimg": img}


# ---------------------------------------------------------------------------
# entry point
# ---------------------------------------------------------------------------

_NC1 = None
_NC2 = None


def _get_ncs():
    global _NC1, _NC2
    if _NC1 is None:
        _NC1 = build_phase1()
        _NC2 = build_phase2()
    return _NC1, _NC2


def kernel(new_knowledge, existing_knowledge, math_metrics,
           W1, b1, W2, b2, W3, b3, Wi1, bi1, Wi2, bi2, Wi3, bi3):
    args = [new_knowledge, existing_knowledge, math_metrics,
            W1, b1, W2, b2, W3, b3, Wi1, bi1, Wi2, bi2, Wi3, bi3]
    (new_knowledge, existing_knowledge, math_metrics,
     W1, b1, W2, b2, W3, b3, Wi1, bi1, Wi2, bi2, Wi3, bi3) = [
        np.asarray(a, np.float32) for a in args]

    nc1, nc2 = _get_ncs()

    x = np.concatenate([new_knowledge[None, :], existing_knowledge], axis=0)
    maps1 = phase1_inputs(x, W1, b1, W2)
    res1 = bass_utils.run_bass_kernel_spmd(
        nc1, maps1, core_ids=list(range(N_CORES)))
    # pure gather: concat per-core z^T blocks -> [128, 176]
    z_full = np.concatenate(
        [res1.results[j]["z_out"] for j in range(N_CORES)], axis=1)

    map2 = phase2_input_maps(z_full, b2, W3, b3, Wi1, bi1, Wi2, bi2, Wi3, bi3,
                             math_metrics)
    res2 = bass_utils.run_bass_kernel_spmd(nc2, [map2], core_ids=[0])
    return res2.results[0]["out"].reshape(()).astype(np.float32)
